# revision 7
# baseline (speedup 1.0000x reference)
"""CPAttention Trainium2 kernel: 8-way batch-data-parallel over 8 NeuronCores.

v3: all-bf16 PE streams via split-precision + sign-matrix score trick.
  - qk proj: 3-term split-bf16 (xhi@whi + xlo@whi + xhi@wlo), fp32 PSUM
  - dots:    3-term split-bf16 of on-device q/k hi/lo splits, 2-head
             quadrant packing (tile_position (0,0)/(64,0))
  - score:   sum_m |d|*mask = sum_m (sign(d)*mask) . d, reduced on the PE
             by streaming T = mask|signbit(d) (exact in bf16) against
             stationary [khiT|kloT]; then score_n = sum_c q_nc * qs_cn
             with fp32 q (vector), ones-matmul partition reduce.
  - softmax: es = (exp(d/8)-1)*mask + 1 -> stream W = (e-1)*mask against
             [V|1]; the +1 parts fold in as K=1 rank-1 matmuls
             ([colsum(V) | 1024] x ones-row) accumulated into the same PSUM.
  - W computed on GpSimd (Pool), exp on Scalar, T/splits on Vector.
Host applies the argsort + 16-step row swap (commutes with w_out).
"""
import numpy as np

import concourse.bacc as bacc
import concourse.tile as tile
from concourse import mybir
from concourse.bass_utils import run_bass_kernel_spmd

F32 = mybir.dt.float32
BF16 = mybir.dt.bfloat16
U16 = mybir.dt.uint16
AOP = mybir.AluOpType
AFT = mybir.ActivationFunctionType

B, N, DIM = 8, 1024, 512
HEADS, DH = 8, 64
INNER = 512
SCALE = DH ** -0.5

_cache = {}


def _build():
    nc = bacc.Bacc()
    xhi_d = nc.declare_dram_parameter("xhi", [DIM, N], BF16, isOutput=False)
    xlo_d = nc.declare_dram_parameter("xlo", [DIM, N], BF16, isOutput=False)
    whi_d = nc.declare_dram_parameter("whi", [DIM, 2 * INNER], BF16, isOutput=False)
    wlo_d = nc.declare_dram_parameter("wlo", [DIM, 2 * INNER], BF16, isOutput=False)
    wv_d = nc.declare_dram_parameter("wv", [DIM, INNER], BF16, isOutput=False)
    wo_d = nc.declare_dram_parameter("wo", [INNER, DIM], BF16, isOutput=False)
    msk_d = nc.declare_dram_parameter("maskT", [N, N], BF16, isOutput=False)
    bout_d = nc.declare_dram_parameter("bout", [1, DIM], F32, isOutput=False)
    idn_d = nc.declare_dram_parameter("idn", [128, 128], BF16, isOutput=False)
    y_out = nc.declare_dram_parameter("y", [N, DIM], F32, isOutput=True)
    sc_out = nc.declare_dram_parameter("score", [1, N], F32, isOutput=True)

    with tile.TileContext(nc) as tc:
        with tc.tile_pool(name="cst", bufs=1) as cst, \
             tc.tile_pool(name="sp", bufs=1) as sp, \
             tc.tile_pool(name="pp", bufs=1, space="PSUM") as pp:

            # ---- loads ----
            xhi = cst.tile([128, 4, N], BF16)
            nc.sync.dma_start(out=xhi, in_=xhi_d[:, :].rearrange("(t p) i -> p t i", p=128))
            xlo = cst.tile([128, 4, N], BF16)
            nc.sync.dma_start(out=xlo, in_=xlo_d[:, :].rearrange("(t p) i -> p t i", p=128))
            whi = cst.tile([128, 4, 2 * INNER], BF16)
            nc.sync.dma_start(out=whi, in_=whi_d[:, :].rearrange("(t p) c -> p t c", p=128))
            wlo = cst.tile([128, 4, 2 * INNER], BF16)
            nc.sync.dma_start(out=wlo, in_=wlo_d[:, :].rearrange("(t p) c -> p t c", p=128))
            wv = cst.tile([128, 4, INNER], BF16)
            nc.sync.dma_start(out=wv, in_=wv_d[:, :].rearrange("(t p) c -> p t c", p=128))
            wo = cst.tile([128, 4, DIM], BF16)
            nc.sync.dma_start(out=wo, in_=wo_d[:, :].rearrange("(t p) e -> p t e", p=128))
            msk = cst.tile([128, 8, N], BF16)
            nc.sync.dma_start(out=msk, in_=msk_d[:, :].rearrange("(t p) i -> p t i", p=128))
            bb = cst.tile([128, DIM], F32)
            nc.sync.dma_start(out=bb, in_=bout_d[0:1, :].to_broadcast([128, DIM]))
            idn = cst.tile([128, 128], BF16)
            nc.sync.dma_start(out=idn, in_=idn_d[:, :])

            ones_bf = cst.tile([128, 1], BF16)
            nc.vector.memset(ones_bf, 1.0)
            ones_f32 = cst.tile([128, 1], F32)
            nc.vector.memset(ones_f32, 1.0)
            onesr1 = cst.tile([1, 64], BF16)
            nc.vector.memset(onesr1, 1.0)
            onesrow = cst.tile([1, 512], BF16)
            nc.vector.memset(onesrow, 1.0)

            qaugA = cst.tile([128, 4, N], F32)
            qaugB = cst.tile([128, 4, N], F32)
            qhi = cst.tile([128, 4, N], BF16)
            qlo = cst.tile([128, 4, N], BF16)
            khi = cst.tile([128, 4, N], BF16)
            klo = cst.tile([128, 4, N], BF16)
            kst = cst.tile([128, 8, 8, 2, 64], BF16)
            vst = cst.tile([128, 8, 8, 65], BF16)
            onorm = cst.tile([128, 4, N], BF16)
            sc_acc = cst.tile([128, N], F32)
            nc.vector.memset(sc_acc, 0.0)
            srow = cst.tile([1, 8, 65], BF16)
            rnz = cst.tile([1, N], F32)
            sc_sb = cst.tile([1, N], F32)

            # ---- phase 1: qk proj (3-term split-bf16) ----
            for ct in range(8):
                for ic in range(2):
                    sl = slice(ic * 512, (ic + 1) * 512)
                    cs = slice(ct * 128, (ct + 1) * 128)
                    pq = pp.tile([128, 512], F32, tag=("dA", "dB")[(ct * 2 + ic) % 2], bufs=2)
                    for kt in range(4):
                        nc.tensor.matmul(pq, whi[:, kt, cs], xhi[:, kt, sl],
                                         start=(kt == 0), stop=False)
                    for kt in range(4):
                        nc.tensor.matmul(pq, whi[:, kt, cs], xlo[:, kt, sl],
                                         start=False, stop=False)
                    for kt in range(4):
                        nc.tensor.matmul(pq, wlo[:, kt, cs], xhi[:, kt, sl],
                                         start=False, stop=(kt == 3))
                    if ct < 4:
                        nc.vector.tensor_copy(qaugA[0:64, ct, sl], pq[0:64, :])
                        nc.vector.tensor_copy(qaugB[64:128, ct, sl], pq[64:128, :])
                        nc.scalar.activation(out=qhi[:, ct, sl], in_=pq, func=AFT.Copy)
                        nc.vector.tensor_tensor(out=qlo[:, ct, sl], in0=pq,
                                                in1=qhi[:, ct, sl], op=AOP.subtract)
                    else:
                        nc.scalar.activation(out=khi[:, ct - 4, sl], in_=pq, func=AFT.Copy)
                        nc.vector.tensor_tensor(out=klo[:, ct - 4, sl], in0=pq,
                                                in1=khi[:, ct - 4, sl], op=AOP.subtract)
                if ct < 4:
                    nc.sync.dma_start(out=qaugA[64:128, ct, :], in_=qaugA[0:64, ct, :])
                    nc.sync.dma_start(out=qaugB[0:64, ct, :], in_=qaugB[64:128, ct, :])

            # ---- phase 2: V proj -> vst [V|1]; k transposes -> kst ----
            nc.vector.memset(vst[:, :, :, 64:65], 1.0)
            for jt in range(8):
                pv = pp.tile([128, 512], F32, tag=("dA", "dB")[jt % 2], bufs=2)
                for kt in range(4):
                    nc.tensor.matmul(pv, xhi[:, kt, jt * 128:(jt + 1) * 128],
                                     wv[:, kt, :], start=(kt == 0), stop=(kt == 3))
                nc.scalar.activation(out=vst[:, jt, :, 0:64],
                                     in_=pv.rearrange("p (h d) -> p h d", h=8),
                                     func=AFT.Copy)

            ttags = ["qsA", "qsB", "avA", "avB"]
            tn = 0
            for ct4 in range(4):
                for jt in range(8):
                    for hl, src in ((0, khi), (1, klo)):
                        tp = pp.tile([128, 128], BF16, tag=ttags[tn % 4], bufs=1)
                        tn += 1
                        nc.tensor.transpose(tp, src[:, ct4, jt * 128:(jt + 1) * 128], idn)
                        nc.vector.tensor_copy(
                            kst[:, jt, 2 * ct4:2 * ct4 + 2, hl, :],
                            tp.rearrange("p (h c) -> p h c", h=2))

            # ---- nnz, colsum(V) ----
            scr = sp.tile([1, 512], F32, tag="scr")
            for ic in range(2):
                sl = slice(ic * 512, (ic + 1) * 512)
                nz = pp.tile([1, 512], F32, tag=ttags[ic], bufs=1)
                for jt in range(8):
                    nc.tensor.matmul(nz, ones_bf, msk[:, jt, sl],
                                     start=(jt == 0), stop=(jt == 7))
                nc.vector.reciprocal_approx_accurate(out=rnz[:, sl], in_=nz, scratch=scr)
            svp = pp.tile([1, 512], F32, tag="avA", bufs=1)
            for jt in range(8):
                nc.tensor.matmul(svp, ones_bf, vst[:, jt, :, 0:64],
                                 start=(jt == 0), stop=(jt == 7))
            nc.vector.tensor_copy(srow[0:1, :, 0:64],
                                  svp.rearrange("p (h d) -> p h d", h=8))
            nc.vector.memset(srow[0:1, :, 64:65], 1024.0)

            # ---- phase 3: attention head pairs ----
            for pr in range(4):
                for ic in range(2):
                    sl = slice(ic * 512, (ic + 1) * 512)
                    qsA = pp.tile([128, 512], F32, tag="qsA", bufs=1)
                    qsB = pp.tile([128, 512], F32, tag="qsB", bufs=1)
                    avA = pp.tile([128, 512], F32, tag="avA", bufs=1)
                    avB = pp.tile([128, 512], F32, tag="avB", bufs=1)
                    nc.tensor.matmul(avA[0:65, :], srow[0:1, 2 * pr, :], onesrow,
                                     start=True, stop=False, skip_group_check=True)
                    nc.tensor.matmul(avB[0:65, :], srow[0:1, 2 * pr + 1, :], onesrow,
                                     start=True, stop=False, skip_group_check=True)
                    for jt in range(8):
                        js = slice(jt * 128, (jt + 1) * 128)
                        dA = pp.tile([128, 512], F32, tag="dA", bufs=2)
                        dB = pp.tile([128, 512], F32, tag="dB", bufs=2)
                        terms = ((khi, qhi), (klo, qhi), (khi, qlo))
                        for t, (st, mv) in enumerate(terms):
                            nc.tensor.matmul(dA, st[0:64, pr, js], mv[0:64, pr, sl],
                                             start=(t == 0), stop=(t == 2),
                                             tile_position=(0, 0),
                                             skip_group_check=True)
                            nc.tensor.matmul(dB, st[64:128, pr, js], mv[64:128, pr, sl],
                                             start=(t == 0), stop=(t == 2),
                                             tile_position=(64, 0),
                                             skip_group_check=True)
                        for h, dd, qs, av in ((0, dA, qsA, avA), (1, dB, qsB, avB)):
                            h8 = 2 * pr + h
                            e = sp.tile([128, 512], BF16, tag="e", bufs=3)
                            nc.scalar.activation(out=e, in_=dd, func=AFT.Exp, scale=SCALE)
                            sg = sp.tile([128, 512], BF16, tag="sg", bufs=3)
                            nc.scalar.activation(out=sg, in_=dd, func=AFT.Sign)
                            T = sp.tile([128, 512], BF16, tag="T", bufs=3)
                            nc.vector.tensor_tensor(out=T, in0=sg,
                                                    in1=msk[:, jt, sl], op=AOP.mult)
                            W = sp.tile([128, 512], BF16, tag="W", bufs=3)
                            nc.vector.scalar_tensor_tensor(
                                out=W, in0=e, scalar=1.0, in1=msk[:, jt, sl],
                                op0=AOP.subtract, op1=AOP.mult)
                            nc.tensor.matmul(qs, kst[:, jt, h8, :, :], T,
                                             start=(jt == 0), stop=(jt == 7),
                                             skip_group_check=True)
                            nc.tensor.matmul(av[0:65, :], vst[:, jt, h8, :], W,
                                             start=False, stop=(jt == 7),
                                             skip_group_check=True)
                    # score assembly: sc_acc += qaug . qs
                    for h, qs, qaug in ((0, qsA, qaugA), (1, qsB, qaugB)):
                        tmp = sp.tile([128, 512], F32, tag="sa", bufs=2)
                        nc.vector.tensor_tensor(out=tmp, in0=qs, in1=qaug[:, pr, sl],
                                                op=AOP.mult)
                        nc.vector.tensor_tensor(out=sc_acc[:, sl], in0=sc_acc[:, sl],
                                                in1=tmp, op=AOP.add)
                    # normalize: onorm = AV * (1/Z) broadcast
                    for h, av in ((0, avA), (1, avB)):
                        zrow = sp.tile([1, 512], F32, tag="zrow", bufs=1)
                        nc.vector.tensor_copy(zrow, av[64:65, :])
                        zr32 = sp.tile([1, 512], F32, tag="zr32", bufs=1)
                        nc.vector.reciprocal_approx_fast(out=zr32, in_=zrow)
                        zrb = sp.tile([1, 512], BF16, tag="zrb", bufs=2)
                        nc.vector.tensor_copy(zrb, zr32)
                        zbc = pp.tile([128, 512], F32, tag=("dA", "dB")[h], bufs=2)
                        nc.tensor.matmul(zbc[0:64, :], onesr1, zrb,
                                         start=True, stop=True, skip_group_check=True)
                        zbs = sp.tile([64, 512], BF16, tag="zbs", bufs=2)
                        nc.vector.tensor_copy(zbs, zbc[0:64, :])
                        if h == 0:
                            nc.vector.tensor_tensor(out=onorm[0:64, pr, sl],
                                                    in0=av[0:64, :], in1=zbs,
                                                    op=AOP.mult)
                        else:
                            otmp = sp.tile([64, 512], BF16, tag="ot", bufs=2)
                            nc.vector.tensor_tensor(out=otmp, in0=av[0:64, :],
                                                    in1=zbs, op=AOP.mult)
                            nc.sync.dma_start(out=onorm[64:128, pr, sl], in_=otmp)

            # ---- phase 4: output projection ----
            for it in range(8):
                yp = pp.tile([128, 512], F32, tag=("dA", "dB")[it % 2], bufs=2)
                for prr in range(4):
                    nc.tensor.matmul(yp, onorm[:, prr, it * 128:(it + 1) * 128],
                                     wo[:, prr, :], start=(prr == 0), stop=(prr == 3))
                yt = sp.tile([128, DIM], F32, tag="yt", bufs=2)
                nc.vector.tensor_tensor(out=yt, in0=yp, in1=bb, op=AOP.add)
                nc.sync.dma_start(out=y_out[it * 128:(it + 1) * 128, :], in_=yt)

            # ---- score finalize ----
            for ic in range(2):
                sl = slice(ic * 512, (ic + 1) * 512)
                scp = pp.tile([1, 512], F32, tag=ttags[ic], bufs=1)
                nc.tensor.matmul(scp, ones_f32, sc_acc[:, sl], start=True, stop=True)
                nc.vector.scalar_tensor_tensor(out=sc_sb[:, sl], in0=scp, scalar=SCALE,
                                               in1=rnz[:, sl], op0=AOP.mult,
                                               op1=AOP.mult)
            nc.gpsimd.dma_start(out=sc_out[:, :], in_=sc_sb)
    nc.finalize()
    return nc


def _get_nc():
    if "nc" not in _cache:
        _cache["nc"] = _build()
    return _cache["nc"]


def _run_device(inputs, trace=False):
    x = np.asarray(inputs["x"], np.float32)
    cp_mask = np.asarray(inputs["cp_mask"])
    w_qkv = np.asarray(inputs["w_qkv"], np.float32)
    w_out = np.asarray(inputs["w_out"], np.float32)
    b_out = np.asarray(inputs["b_out"], np.float32)

    bf = mybir.dt.np(BF16)
    wqk = np.ascontiguousarray(w_qkv[:, :2 * INNER])
    whi = wqk.astype(bf)
    wlo = (wqk - whi.astype(np.float32)).astype(bf)
    wv = np.ascontiguousarray(w_qkv[:, 2 * INNER:]).astype(bf)
    wob = np.ascontiguousarray(w_out).astype(bf)
    maskT = np.ascontiguousarray(cp_mask.T).astype(bf)
    boutr = np.ascontiguousarray(b_out.reshape(1, DIM))
    idn = np.eye(128, dtype=bf)

    in_maps = []
    for b in range(B):
        xT = np.ascontiguousarray(x[b].T)
        xhi = xT.astype(bf)
        xlo = (xT - xhi.astype(np.float32)).astype(bf)
        in_maps.append({
            "xhi": xhi, "xlo": xlo,
            "whi": whi, "wlo": wlo,
            "wv": wv, "wo": wob,
            "maskT": maskT, "bout": boutr, "idn": idn,
        })

    nc = _get_nc()
    res = run_bass_kernel_spmd(nc, in_maps, core_ids=list(range(B)), trace=trace)
    y = np.stack([res.results[b]["y"] for b in range(B)])
    score = np.stack([res.results[b]["score"][0] for b in range(B)])
    return y, score, res


def _apply_swap(y, score, patches):
    idx = np.argsort(score, axis=-1, kind="stable")[::-1]
    out = y.copy()
    clone = y
    bi = np.arange(B)
    for i in range(1, patches + 1):
        ti = idx[:, i]
        out[bi, i] = clone[bi, ti]
        out[bi, ti] = clone[:, i]
    return out


def kernel(**inputs):
    patches = int(np.asarray(inputs["patches_in_core_nodes"]))
    y, score, _ = _run_device(inputs, trace=False)
    return _apply_swap(y, score, patches)


# revision 17
# speedup vs baseline: 1.4890x; 1.4890x over previous
"""CPAttention Trainium2 kernel: 8-way batch-data-parallel over 8 NeuronCores.

v3: all-bf16 PE streams via split-precision + sign-matrix score trick.
  - qk proj: 3-term split-bf16 (xhi@whi + xlo@whi + xhi@wlo), fp32 PSUM
  - dots:    3-term split-bf16 of on-device q/k hi/lo splits, 2-head
             quadrant packing (tile_position (0,0)/(64,0))
  - score:   sum_m |d|*mask = sum_m (sign(d)*mask) . d, reduced on the PE
             by streaming T = mask|signbit(d) (exact in bf16) against
             stationary [khiT|kloT]; then score_n = sum_c q_nc * qs_cn
             with fp32 q (vector), ones-matmul partition reduce.
  - softmax: es = (exp(d/8)-1)*mask + 1 -> stream W = (e-1)*mask against
             [V|1]; the +1 parts fold in as K=1 rank-1 matmuls
             ([colsum(V) | 1024] x ones-row) accumulated into the same PSUM.
  - W computed on GpSimd (Pool), exp on Scalar, T/splits on Vector.
Host applies the argsort + 16-step row swap (commutes with w_out).
"""
import numpy as np

import concourse.bacc as bacc
import concourse.tile as tile
from concourse import mybir
from concourse.bass_utils import run_bass_kernel_spmd

F32 = mybir.dt.float32
BF16 = mybir.dt.bfloat16
U16 = mybir.dt.uint16
AOP = mybir.AluOpType
AFT = mybir.ActivationFunctionType

B, N, DIM = 8, 1024, 512
HEADS, DH = 8, 64
INNER = 512
SCALE = DH ** -0.5

_cache = {}


def _build():
    nc = bacc.Bacc()
    xhi_d = nc.declare_dram_parameter("xhi", [DIM, N], BF16, isOutput=False)
    xlo_d = nc.declare_dram_parameter("xlo", [DIM, N], BF16, isOutput=False)
    whi_d = nc.declare_dram_parameter("whi", [DIM, 2 * INNER], BF16, isOutput=False)
    wlo_d = nc.declare_dram_parameter("wlo", [DIM, 2 * INNER], BF16, isOutput=False)
    wv_d = nc.declare_dram_parameter("wv", [DIM, INNER], BF16, isOutput=False)
    wo_d = nc.declare_dram_parameter("wo", [INNER, DIM], BF16, isOutput=False)
    msk_d = nc.declare_dram_parameter("maskT", [N, N], BF16, isOutput=False)
    bout_d = nc.declare_dram_parameter("bout", [1, DIM], F32, isOutput=False)
    idn_d = nc.declare_dram_parameter("idn", [128, 128], BF16, isOutput=False)
    y_out = nc.declare_dram_parameter("y", [N, DIM], F32, isOutput=True)
    sc_out = nc.declare_dram_parameter("score", [1, N], F32, isOutput=True)

    with tile.TileContext(nc) as tc:
        with tc.tile_pool(name="cst", bufs=1) as cst, \
             tc.tile_pool(name="sp", bufs=1) as sp, \
             tc.tile_pool(name="pp", bufs=1, space="PSUM") as pp:

            # ---- loads ----
            xhi = cst.tile([128, 4, N], BF16)
            nc.sync.dma_start(out=xhi, in_=xhi_d[:, :].rearrange("(t p) i -> p t i", p=128))
            xlo = cst.tile([128, 4, N], BF16)
            nc.sync.dma_start(out=xlo, in_=xlo_d[:, :].rearrange("(t p) i -> p t i", p=128))
            whi = cst.tile([128, 4, 2 * INNER], BF16)
            nc.sync.dma_start(out=whi, in_=whi_d[:, :].rearrange("(t p) c -> p t c", p=128))
            wlo = cst.tile([128, 4, 2 * INNER], BF16)
            nc.sync.dma_start(out=wlo, in_=wlo_d[:, :].rearrange("(t p) c -> p t c", p=128))
            wv = cst.tile([128, 4, INNER], BF16)
            nc.sync.dma_start(out=wv, in_=wv_d[:, :].rearrange("(t p) c -> p t c", p=128))
            wo = cst.tile([128, 4, DIM], BF16)
            nc.sync.dma_start(out=wo, in_=wo_d[:, :].rearrange("(t p) e -> p t e", p=128))
            msk = cst.tile([128, 8, N], BF16)
            nc.sync.dma_start(out=msk, in_=msk_d[:, :].rearrange("(t p) i -> p t i", p=128))
            bb = cst.tile([128, DIM], F32)
            nc.sync.dma_start(out=bb, in_=bout_d[0:1, :].to_broadcast([128, DIM]))
            idn = cst.tile([128, 128], BF16)
            nc.sync.dma_start(out=idn, in_=idn_d[:, :])

            ones_bf = cst.tile([128, 1], BF16)
            nc.vector.memset(ones_bf, 1.0)
            ones_f32 = cst.tile([128, 1], F32)
            nc.vector.memset(ones_f32, 1.0)
            onesr1 = cst.tile([1, 64], BF16)
            nc.vector.memset(onesr1, 1.0)

            qaugA = cst.tile([128, 4, N], F32)
            qaugB = cst.tile([128, 4, N], F32)
            qhi = cst.tile([128, 4, N], BF16)
            qlo = cst.tile([128, 4, N], BF16)
            khi = cst.tile([128, 4, N], BF16)
            klo = cst.tile([128, 4, N], BF16)
            kst = cst.tile([128, 8, 8, 2, 64], BF16)
            vst = cst.tile([128, 8, 8, 65], BF16)
            onorm = cst.tile([128, 4, N], BF16)
            sc_acc = cst.tile([128, N], F32)
            nc.vector.memset(sc_acc, 0.0)
            rnz = cst.tile([1, N], F32)
            sc_sb = cst.tile([1, N], F32)

            # ---- phase 1: qk proj (3-term split-bf16) ----
            for ct in range(8):
                for ic in range(2):
                    sl = slice(ic * 512, (ic + 1) * 512)
                    cs = slice(ct * 128, (ct + 1) * 128)
                    pq = pp.tile([128, 512], F32, tag=("dA", "dB")[(ct * 2 + ic) % 2], bufs=2)
                    for kt in range(4):
                        nc.tensor.matmul(pq, whi[:, kt, cs], xhi[:, kt, sl],
                                         start=(kt == 0), stop=False)
                    for kt in range(4):
                        nc.tensor.matmul(pq, whi[:, kt, cs], xlo[:, kt, sl],
                                         start=False, stop=False)
                    for kt in range(4):
                        nc.tensor.matmul(pq, wlo[:, kt, cs], xhi[:, kt, sl],
                                         start=False, stop=(kt == 3))
                    if ct < 4:
                        nc.vector.tensor_copy(qaugA[0:64, ct, sl], pq[0:64, :])
                        nc.vector.tensor_copy(qaugB[64:128, ct, sl], pq[64:128, :])
                        nc.scalar.activation(out=qhi[:, ct, sl], in_=pq, func=AFT.Copy)
                        nc.vector.tensor_tensor(out=qlo[:, ct, sl], in0=pq,
                                                in1=qhi[:, ct, sl], op=AOP.subtract)
                    else:
                        nc.scalar.activation(out=khi[:, ct - 4, sl], in_=pq, func=AFT.Copy)
                        nc.vector.tensor_tensor(out=klo[:, ct - 4, sl], in0=pq,
                                                in1=khi[:, ct - 4, sl], op=AOP.subtract)
                if ct < 4:
                    nc.sync.dma_start(out=qaugA[64:128, ct, :], in_=qaugA[0:64, ct, :])
                    nc.sync.dma_start(out=qaugB[0:64, ct, :], in_=qaugB[64:128, ct, :])

            # ---- phase 2: V proj -> vst [V|1]; k transposes -> kst ----
            nc.vector.memset(vst[:, :, :, 64:65], 1.0)
            for jt in range(8):
                pv = pp.tile([128, 512], F32, tag=("dA", "dB")[jt % 2], bufs=2)
                for kt in range(4):
                    nc.tensor.matmul(pv, xhi[:, kt, jt * 128:(jt + 1) * 128],
                                     wv[:, kt, :], start=(kt == 0), stop=(kt == 3))
                nc.scalar.activation(out=vst[:, jt, :, 0:64],
                                     in_=pv.rearrange("p (h d) -> p h d", h=8),
                                     func=AFT.Copy)

            ttags = ["qsA", "qsB", "avA", "avB"]
            tn = 0
            for ct4 in range(4):
                for jt in range(8):
                    for hl, src in ((0, khi), (1, klo)):
                        tp = pp.tile([128, 128], BF16, tag=ttags[tn % 4], bufs=1)
                        tn += 1
                        nc.tensor.transpose(tp, src[:, ct4, jt * 128:(jt + 1) * 128], idn)
                        nc.vector.tensor_copy(
                            kst[:, jt, 2 * ct4:2 * ct4 + 2, hl, :],
                            tp.rearrange("p (h c) -> p h c", h=2))

            # ---- nnz, colsum(V) ----
            scr = sp.tile([1, 512], F32, tag="sa")
            for ic in range(2):
                sl = slice(ic * 512, (ic + 1) * 512)
                nz = pp.tile([1, 512], F32, tag=ttags[ic], bufs=1)
                for jt in range(8):
                    nc.tensor.matmul(nz, ones_bf, msk[:, jt, sl],
                                     start=(jt == 0), stop=(jt == 7))
                nc.vector.reciprocal_approx_accurate(out=rnz[:, sl], in_=nz, scratch=scr)

            # ---- phase 3: attention head pairs (carry-pipelined, es-form) ----
            pend_early = []
            pend_late = []

            def emit(lst):
                while lst:
                    lst.pop(0)()

            for pr in range(4):
                for ic in range(2):
                    sl = slice(ic * 512, (ic + 1) * 512)
                    qsA = pp.tile([128, 512], F32, tag="qsA", bufs=1)
                    qsB = pp.tile([128, 512], F32, tag="qsB", bufs=1)
                    avA = pp.tile([128, 512], F32, tag="avA", bufs=1)
                    avB = pp.tile([128, 512], F32, tag="avB", bufs=1)
                    qs_av = ((qsA, avA), (qsB, avB))
                    prev = None
                    for jp in range(4):
                        rset = []
                        for h in range(2):
                            tr = sp.tile([128, 2, 512], BF16, tag=f"tr{h}", bufs=2)
                            er = sp.tile([128, 2, 512], BF16, tag=f"er{h}", bufs=2)
                            gr = sp.tile([128, 2, 512], BF16, tag=f"gr{h}", bufs=1)
                            Tr = sp.tile([128, 2, 512], BF16, tag=f"Tr{h}", bufs=2)
                            rset.append((tr, er, gr, Tr))
                        dds = []
                        for j2 in range(2):
                            jt = 2 * jp + j2
                            js = slice(jt * 128, (jt + 1) * 128)
                            dA = pp.tile([128, 512], F32, tag="dA", bufs=2)
                            dB = pp.tile([128, 512], F32, tag="dB", bufs=2)
                            terms = ((khi, qhi), (klo, qhi), (khi, qlo))
                            for tix, (st, mv) in enumerate(terms):
                                nc.tensor.matmul(dA, st[0:64, pr, js], mv[0:64, pr, sl],
                                                 start=(tix == 0), stop=(tix == 2),
                                                 tile_position=(0, 0),
                                                 skip_group_check=True)
                                nc.tensor.matmul(dB, st[64:128, pr, js], mv[64:128, pr, sl],
                                                 start=(tix == 0), stop=(tix == 2),
                                                 tile_position=(64, 0),
                                                 skip_group_check=True)
                            if jp == 0 and j2 == 1:
                                emit(pend_early)
                            if jp == 1 and j2 == 0:
                                emit(pend_late)
                            for h, dd in ((0, dA), (1, dB)):
                                nc.vector.tensor_tensor(out=rset[h][0][:, j2, :],
                                                        in0=dd, in1=msk[:, jt, sl],
                                                        op=AOP.mult)
                            dds.append((dA, dB))
                        m2 = msk[:, 2 * jp:2 * jp + 2, sl]
                        for h in range(2):
                            tr, er, gr, Tr = rset[h]
                            nc.scalar.activation(out=er, in_=tr, func=AFT.Exp,
                                                 scale=SCALE)
                            nc.gpsimd.tensor_scalar(out=gr, in0=tr, scalar1=0.0,
                                                    scalar2=-2.0, op0=AOP.is_lt,
                                                    op1=AOP.mult)
                            nc.gpsimd.tensor_tensor(out=Tr, in0=gr, in1=m2,
                                                    op=AOP.add)
                        if prev is not None:
                            pjp, prset = prev
                            for h in range(2):
                                qs, av = qs_av[h]
                                h8 = 2 * pr + h
                                for j2 in range(2):
                                    jt = 2 * pjp + j2
                                    nc.tensor.matmul(qs, kst[:, jt, h8, :, :],
                                                     prset[h][3][:, j2, :],
                                                     start=(jt == 0), stop=False,
                                                     skip_group_check=True)
                                    nc.tensor.matmul(av[0:65, :], vst[:, jt, h8, :],
                                                     prset[h][1][:, j2, :],
                                                     start=(jt == 0), stop=False,
                                                     skip_group_check=True)
                        prev = (jp, rset)
                    pjp, prset = prev
                    for h in range(2):
                        qs, av = qs_av[h]
                        h8 = 2 * pr + h
                        for j2 in range(2):
                            jt = 2 * pjp + j2
                            nc.tensor.matmul(qs, kst[:, jt, h8, :, :],
                                             prset[h][3][:, j2, :],
                                             start=False, stop=(j2 == 1),
                                             skip_group_check=True)
                            nc.tensor.matmul(av[0:65, :], vst[:, jt, h8, :],
                                             prset[h][1][:, j2, :],
                                             start=False, stop=(j2 == 1),
                                             skip_group_check=True)
                    # spill qs to SBUF (frees PSUM fast); assembly deferred
                    qss = sp.tile([128, 2, 512], F32, tag="qss", bufs=1)
                    nc.vector.tensor_copy(qss[:, 0, :], qs_av[0][0])
                    nc.vector.tensor_copy(qss[:, 1, :], qs_av[1][0])

                    def assy(pr=pr, sl=sl, qss=qss):
                        for h, qaug in ((0, qaugA), (1, qaugB)):
                            tmp = sp.tile([128, 512], F32, tag="sa", bufs=1,
                                          name="tmp")
                            nc.vector.tensor_tensor(out=tmp, in0=qss[:, h, :],
                                                    in1=qaug[:, pr, sl], op=AOP.mult)
                            nc.vector.tensor_tensor(out=sc_acc[:, sl],
                                                    in0=sc_acc[:, sl],
                                                    in1=tmp, op=AOP.add)
                    pend_late.append(assy)
                    # free AV psum via DMA, defer normalize into next pair
                    for h in range(2):
                        av = qs_av[h][1]
                        avs = sp.tile([65, 512], F32, tag=f"avs{h}", bufs=1)
                        nc.scalar.activation(out=avs, in_=av[0:65, :], func=AFT.Copy)
                        zrow = sp.tile([1, 512], F32, tag="zrow", bufs=1)
                        nc.vector.tensor_copy(zrow, avs[64:65, :])
                        zr32 = sp.tile([1, 512], F32, tag="zr32", bufs=1)
                        nc.vector.reciprocal_approx_fast(out=zr32, in_=zrow)
                        zrb = sp.tile([1, 512], BF16, tag=f"zrb{h}", bufs=1)
                        nc.vector.tensor_copy(zrb, zr32)

                        def norm(h=h, pr=pr, ic=ic, sl=sl, avs=avs, zrb=zrb):
                            zbc = pp.tile([128, 512], F32, tag=("qsA", "qsB")[h],
                                          bufs=1, name=f"zbc{h}")
                            nc.tensor.matmul(zbc[0:64, :], onesr1, zrb,
                                             start=True, stop=True,
                                             skip_group_check=True)
                            if h == 0:
                                nc.vector.tensor_tensor(out=onorm[0:64, pr, sl],
                                                        in0=avs[0:64, :],
                                                        in1=zbc[0:64, :], op=AOP.mult)
                            else:
                                otmp = sp.tile([64, 512], BF16, tag="zr32", bufs=1,
                                               name="otmp")
                                nc.vector.tensor_tensor(out=otmp, in0=avs[0:64, :],
                                                        in1=zbc[0:64, :], op=AOP.mult)
                                nc.sync.dma_start(out=onorm[64:128, pr, sl], in_=otmp)
                        pend_early.append(norm)
            emit(pend_early)
            emit(pend_late)

            # ---- phase 4: output projection ----
            for it in range(8):
                yp = pp.tile([128, 512], F32, tag=("dA", "dB")[it % 2], bufs=2)
                for prr in range(4):
                    nc.tensor.matmul(yp, onorm[:, prr, it * 128:(it + 1) * 128],
                                     wo[:, prr, :], start=(prr == 0), stop=(prr == 3))
                yt = sp.tile([128, DIM], F32, tag="qss", bufs=1)
                nc.vector.tensor_tensor(out=yt, in0=yp, in1=bb, op=AOP.add)
                nc.sync.dma_start(out=y_out[it * 128:(it + 1) * 128, :], in_=yt)

            # ---- score finalize ----
            for ic in range(2):
                sl = slice(ic * 512, (ic + 1) * 512)
                scp = pp.tile([1, 512], F32, tag=ttags[ic], bufs=1)
                nc.tensor.matmul(scp, ones_f32, sc_acc[:, sl], start=True, stop=True)
                nc.vector.scalar_tensor_tensor(out=sc_sb[:, sl], in0=scp, scalar=SCALE,
                                               in1=rnz[:, sl], op0=AOP.mult,
                                               op1=AOP.mult)
            nc.gpsimd.dma_start(out=sc_out[:, :], in_=sc_sb)
    nc.finalize()
    return nc


def _get_nc():
    if "nc" not in _cache:
        _cache["nc"] = _build()
    return _cache["nc"]


def _run_device(inputs, trace=False):
    x = np.asarray(inputs["x"], np.float32)
    cp_mask = np.asarray(inputs["cp_mask"])
    w_qkv = np.asarray(inputs["w_qkv"], np.float32)
    w_out = np.asarray(inputs["w_out"], np.float32)
    b_out = np.asarray(inputs["b_out"], np.float32)

    bf = mybir.dt.np(BF16)
    wqk = np.ascontiguousarray(w_qkv[:, :2 * INNER])
    whi = wqk.astype(bf)
    wlo = (wqk - whi.astype(np.float32)).astype(bf)
    wv = np.ascontiguousarray(w_qkv[:, 2 * INNER:]).astype(bf)
    wob = np.ascontiguousarray(w_out).astype(bf)
    maskT = np.ascontiguousarray(cp_mask.T).astype(bf)
    boutr = np.ascontiguousarray(b_out.reshape(1, DIM))
    idn = np.eye(128, dtype=bf)

    in_maps = []
    for b in range(B):
        xT = np.ascontiguousarray(x[b].T)
        xhi = xT.astype(bf)
        xlo = (xT - xhi.astype(np.float32)).astype(bf)
        in_maps.append({
            "xhi": xhi, "xlo": xlo,
            "whi": whi, "wlo": wlo,
            "wv": wv, "wo": wob,
            "maskT": maskT, "bout": boutr, "idn": idn,
        })

    nc = _get_nc()
    res = run_bass_kernel_spmd(nc, in_maps, core_ids=list(range(B)), trace=trace)
    y = np.stack([res.results[b]["y"] for b in range(B)])
    score = np.stack([res.results[b]["score"][0] for b in range(B)])
    return y, score, res


def _apply_swap(y, score, patches):
    idx = np.argsort(score, axis=-1, kind="stable")[::-1]
    out = y.copy()
    clone = y
    bi = np.arange(B)
    for i in range(1, patches + 1):
        ti = idx[:, i]
        out[bi, i] = clone[bi, ti]
        out[bi, ti] = clone[:, i]
    return out


def kernel(**inputs):
    patches = int(np.asarray(inputs["patches_in_core_nodes"]))
    y, score, _ = _run_device(inputs, trace=False)
    return _apply_swap(y, score, patches)


# revision 18
# speedup vs baseline: 1.5132x; 1.0163x over previous
"""CPAttention Trainium2 kernel: 8-way batch-data-parallel over 8 NeuronCores.

v3: all-bf16 PE streams via split-precision + sign-matrix score trick.
  - qk proj: 3-term split-bf16 (xhi@whi + xlo@whi + xhi@wlo), fp32 PSUM
  - dots:    3-term split-bf16 of on-device q/k hi/lo splits, 2-head
             quadrant packing (tile_position (0,0)/(64,0))
  - score:   sum_m |d|*mask = sum_m (sign(d)*mask) . d, reduced on the PE
             by streaming T = mask|signbit(d) (exact in bf16) against
             stationary [khiT|kloT]; then score_n = sum_c q_nc * qs_cn
             with fp32 q (vector), ones-matmul partition reduce.
  - softmax: es = (exp(d/8)-1)*mask + 1 -> stream W = (e-1)*mask against
             [V|1]; the +1 parts fold in as K=1 rank-1 matmuls
             ([colsum(V) | 1024] x ones-row) accumulated into the same PSUM.
  - W computed on GpSimd (Pool), exp on Scalar, T/splits on Vector.
Host applies the argsort + 16-step row swap (commutes with w_out).
"""
import numpy as np

import concourse.bacc as bacc
import concourse.tile as tile
from concourse import mybir
from concourse.bass_utils import run_bass_kernel_spmd

F32 = mybir.dt.float32
BF16 = mybir.dt.bfloat16
U16 = mybir.dt.uint16
AOP = mybir.AluOpType
AFT = mybir.ActivationFunctionType

B, N, DIM = 8, 1024, 512
HEADS, DH = 8, 64
INNER = 512
SCALE = DH ** -0.5

_cache = {}


def _build():
    nc = bacc.Bacc()
    xhi_d = nc.declare_dram_parameter("xhi", [DIM, N], BF16, isOutput=False)
    xlo_d = nc.declare_dram_parameter("xlo", [DIM, N], BF16, isOutput=False)
    whi_d = nc.declare_dram_parameter("whi", [DIM, 2 * INNER], BF16, isOutput=False)
    wlo_d = nc.declare_dram_parameter("wlo", [DIM, 2 * INNER], BF16, isOutput=False)
    wv_d = nc.declare_dram_parameter("wv", [DIM, INNER], BF16, isOutput=False)
    wo_d = nc.declare_dram_parameter("wo", [INNER, DIM], BF16, isOutput=False)
    msk_d = nc.declare_dram_parameter("maskT", [N, N], BF16, isOutput=False)
    bout_d = nc.declare_dram_parameter("bout", [1, DIM], F32, isOutput=False)
    idn_d = nc.declare_dram_parameter("idn", [128, 128], BF16, isOutput=False)
    y_out = nc.declare_dram_parameter("y", [N, DIM], F32, isOutput=True)
    sc_out = nc.declare_dram_parameter("score", [1, N], F32, isOutput=True)

    with tile.TileContext(nc) as tc:
        with tc.tile_pool(name="cst", bufs=1) as cst, \
             tc.tile_pool(name="sp", bufs=1) as sp, \
             tc.tile_pool(name="pp", bufs=1, space="PSUM") as pp:

            # ---- loads ----
            xhi = cst.tile([128, 4, N], BF16)
            nc.sync.dma_start(out=xhi, in_=xhi_d[:, :].rearrange("(t p) i -> p t i", p=128))
            whi = cst.tile([128, 4, 2 * INNER], BF16)
            nc.sync.dma_start(out=whi, in_=whi_d[:, :].rearrange("(t p) c -> p t c", p=128))
            xlo = cst.tile([128, 4, N], BF16)
            nc.sync.dma_start(out=xlo, in_=xlo_d[:, :].rearrange("(t p) i -> p t i", p=128))
            wlo = cst.tile([128, 4, 2 * INNER], BF16)
            nc.sync.dma_start(out=wlo, in_=wlo_d[:, :].rearrange("(t p) c -> p t c", p=128))
            wv = cst.tile([128, 4, INNER], BF16)
            nc.sync.dma_start(out=wv, in_=wv_d[:, :].rearrange("(t p) c -> p t c", p=128))
            wo = cst.tile([128, 4, DIM], BF16)
            nc.sync.dma_start(out=wo, in_=wo_d[:, :].rearrange("(t p) e -> p t e", p=128))
            msk = cst.tile([128, 8, N], BF16)
            nc.sync.dma_start(out=msk, in_=msk_d[:, :].rearrange("(t p) i -> p t i", p=128))
            bb = cst.tile([128, DIM], F32)
            nc.sync.dma_start(out=bb, in_=bout_d[0:1, :].to_broadcast([128, DIM]))
            idn = cst.tile([128, 128], BF16)
            nc.sync.dma_start(out=idn, in_=idn_d[:, :])

            ones_bf = cst.tile([128, 1], BF16)
            nc.vector.memset(ones_bf, 1.0)
            ones_f32 = cst.tile([128, 1], F32)
            nc.vector.memset(ones_f32, 1.0)
            onesr1 = cst.tile([1, 64], BF16)
            nc.vector.memset(onesr1, 1.0)

            qaugA = cst.tile([128, 4, N], F32)
            qaugB = cst.tile([128, 4, N], F32)
            qhi = cst.tile([128, 4, N], BF16)
            qlo = cst.tile([128, 4, N], BF16)
            khi = cst.tile([128, 4, N], BF16)
            klo = cst.tile([128, 4, N], BF16)
            kst = cst.tile([128, 8, 8, 2, 64], BF16)
            vst = cst.tile([128, 8, 8, 65], BF16)
            onorm = cst.tile([128, 4, N], BF16)
            sc_acc = cst.tile([128, N], F32)
            nc.vector.memset(sc_acc, 0.0)
            rnz = cst.tile([1, N], F32)
            sc_sb = cst.tile([1, N], F32)

            # ---- phase 1: qk proj (3-term split-bf16) ----
            for ct in range(8):
                for ic in range(2):
                    sl = slice(ic * 512, (ic + 1) * 512)
                    cs = slice(ct * 128, (ct + 1) * 128)
                    pq = pp.tile([128, 512], F32, tag=("dA", "dB")[(ct * 2 + ic) % 2], bufs=2)
                    for kt in range(4):
                        nc.tensor.matmul(pq, whi[:, kt, cs], xhi[:, kt, sl],
                                         start=(kt == 0), stop=False)
                    for kt in range(4):
                        nc.tensor.matmul(pq, whi[:, kt, cs], xlo[:, kt, sl],
                                         start=False, stop=False)
                    for kt in range(4):
                        nc.tensor.matmul(pq, wlo[:, kt, cs], xhi[:, kt, sl],
                                         start=False, stop=(kt == 3))
                    if ct < 4:
                        nc.vector.tensor_copy(qaugA[0:64, ct, sl], pq[0:64, :])
                        nc.vector.tensor_copy(qaugB[64:128, ct, sl], pq[64:128, :])
                        nc.scalar.activation(out=qhi[:, ct, sl], in_=pq, func=AFT.Copy)
                        nc.vector.tensor_tensor(out=qlo[:, ct, sl], in0=pq,
                                                in1=qhi[:, ct, sl], op=AOP.subtract)
                    else:
                        nc.scalar.activation(out=khi[:, ct - 4, sl], in_=pq, func=AFT.Copy)
                        nc.vector.tensor_tensor(out=klo[:, ct - 4, sl], in0=pq,
                                                in1=khi[:, ct - 4, sl], op=AOP.subtract)
                if ct < 4:
                    nc.sync.dma_start(out=qaugA[64:128, ct, :], in_=qaugA[0:64, ct, :])
                    nc.sync.dma_start(out=qaugB[0:64, ct, :], in_=qaugB[64:128, ct, :])

            # ---- phase 2: V proj -> vst [V|1]; k transposes -> kst ----
            nc.vector.memset(vst[:, :, :, 64:65], 1.0)
            for jt in range(8):
                pv = pp.tile([128, 512], F32, tag=("dA", "dB")[jt % 2], bufs=2)
                for kt in range(4):
                    nc.tensor.matmul(pv, xhi[:, kt, jt * 128:(jt + 1) * 128],
                                     wv[:, kt, :], start=(kt == 0), stop=(kt == 3))
                nc.scalar.activation(out=vst[:, jt, :, 0:64],
                                     in_=pv.rearrange("p (h d) -> p h d", h=8),
                                     func=AFT.Copy)

            ttags = ["qsA", "qsB", "avA", "avB"]
            tn = 0
            for ct4 in range(4):
                for jt in range(8):
                    for hl, src in ((0, khi), (1, klo)):
                        tp = pp.tile([128, 128], BF16, tag=ttags[tn % 4], bufs=1)
                        tn += 1
                        nc.tensor.transpose(tp, src[:, ct4, jt * 128:(jt + 1) * 128], idn)
                        nc.vector.tensor_copy(
                            kst[:, jt, 2 * ct4:2 * ct4 + 2, hl, :],
                            tp.rearrange("p (h c) -> p h c", h=2))

            # ---- nnz, colsum(V) ----
            scr = sp.tile([1, 512], F32, tag="sa")
            for ic in range(2):
                sl = slice(ic * 512, (ic + 1) * 512)
                nz = pp.tile([1, 512], F32, tag=ttags[ic], bufs=1)
                for jt in range(8):
                    nc.tensor.matmul(nz, ones_bf, msk[:, jt, sl],
                                     start=(jt == 0), stop=(jt == 7))
                nc.vector.reciprocal_approx_accurate(out=rnz[:, sl], in_=nz, scratch=scr)

            # ---- phase 3: attention head pairs (carry-pipelined, es-form) ----
            pend_early = []
            pend_late = []

            def emit(lst):
                while lst:
                    lst.pop(0)()

            for pr in range(4):
                for ic in range(2):
                    sl = slice(ic * 512, (ic + 1) * 512)
                    qsA = pp.tile([128, 512], F32, tag="qsA", bufs=1)
                    qsB = pp.tile([128, 512], F32, tag="qsB", bufs=1)
                    avA = pp.tile([128, 512], F32, tag="avA", bufs=1)
                    avB = pp.tile([128, 512], F32, tag="avB", bufs=1)
                    qs_av = ((qsA, avA), (qsB, avB))
                    prev = None
                    for jp in range(4):
                        rset = []
                        for h in range(2):
                            tr = sp.tile([128, 2, 512], BF16, tag=f"tr{h}", bufs=2)
                            er = sp.tile([128, 2, 512], BF16, tag=f"er{h}", bufs=2)
                            gr = sp.tile([128, 2, 512], BF16, tag=f"gr{h}", bufs=1)
                            Tr = sp.tile([128, 2, 512], BF16, tag=f"Tr{h}", bufs=2)
                            rset.append((tr, er, gr, Tr))
                        dds = []
                        for j2 in range(2):
                            jt = 2 * jp + j2
                            js = slice(jt * 128, (jt + 1) * 128)
                            dA = pp.tile([128, 512], F32, tag="dA", bufs=2)
                            dB = pp.tile([128, 512], F32, tag="dB", bufs=2)
                            terms = ((khi, qhi), (klo, qhi), (khi, qlo))
                            for tix, (st, mv) in enumerate(terms):
                                nc.tensor.matmul(dA, st[0:64, pr, js], mv[0:64, pr, sl],
                                                 start=(tix == 0), stop=(tix == 2),
                                                 tile_position=(0, 0),
                                                 skip_group_check=True)
                                nc.tensor.matmul(dB, st[64:128, pr, js], mv[64:128, pr, sl],
                                                 start=(tix == 0), stop=(tix == 2),
                                                 tile_position=(64, 0),
                                                 skip_group_check=True)
                            if jp == 1 and j2 == 0:
                                emit(pend_early)
                            if jp == 1 and j2 == 1:
                                emit(pend_late)
                            for h, dd in ((0, dA), (1, dB)):
                                nc.vector.tensor_tensor(out=rset[h][0][:, j2, :],
                                                        in0=dd, in1=msk[:, jt, sl],
                                                        op=AOP.mult)
                            dds.append((dA, dB))
                        m2 = msk[:, 2 * jp:2 * jp + 2, sl]
                        for h in range(2):
                            tr, er, gr, Tr = rset[h]
                            nc.scalar.activation(out=er, in_=tr, func=AFT.Exp,
                                                 scale=SCALE)
                            nc.gpsimd.tensor_scalar(out=gr, in0=tr, scalar1=0.0,
                                                    scalar2=-2.0, op0=AOP.is_lt,
                                                    op1=AOP.mult)
                            nc.gpsimd.tensor_tensor(out=Tr, in0=gr, in1=m2,
                                                    op=AOP.add)
                        if prev is not None:
                            pjp, prset = prev
                            for h in range(2):
                                qs, av = qs_av[h]
                                h8 = 2 * pr + h
                                for j2 in range(2):
                                    jt = 2 * pjp + j2
                                    nc.tensor.matmul(qs, kst[:, jt, h8, :, :],
                                                     prset[h][3][:, j2, :],
                                                     start=(jt == 0), stop=False,
                                                     skip_group_check=True)
                                    nc.tensor.matmul(av[0:65, :], vst[:, jt, h8, :],
                                                     prset[h][1][:, j2, :],
                                                     start=(jt == 0), stop=False,
                                                     skip_group_check=True)
                        prev = (jp, rset)
                    pjp, prset = prev
                    for h in range(2):
                        qs, av = qs_av[h]
                        h8 = 2 * pr + h
                        for j2 in range(2):
                            jt = 2 * pjp + j2
                            nc.tensor.matmul(qs, kst[:, jt, h8, :, :],
                                             prset[h][3][:, j2, :],
                                             start=False, stop=(j2 == 1),
                                             skip_group_check=True)
                            nc.tensor.matmul(av[0:65, :], vst[:, jt, h8, :],
                                             prset[h][1][:, j2, :],
                                             start=False, stop=(j2 == 1),
                                             skip_group_check=True)
                    # spill qs to SBUF (frees PSUM fast); assembly deferred
                    qss = sp.tile([128, 2, 512], F32, tag="qss", bufs=1)
                    nc.vector.tensor_copy(qss[:, 0, :], qs_av[0][0])
                    nc.vector.tensor_copy(qss[:, 1, :], qs_av[1][0])

                    def assy(pr=pr, sl=sl, qss=qss):
                        for h, qaug in ((0, qaugA), (1, qaugB)):
                            tmp = sp.tile([128, 512], F32, tag="sa", bufs=1,
                                          name="tmp")
                            nc.vector.tensor_tensor(out=tmp, in0=qss[:, h, :],
                                                    in1=qaug[:, pr, sl], op=AOP.mult)
                            nc.vector.tensor_tensor(out=sc_acc[:, sl],
                                                    in0=sc_acc[:, sl],
                                                    in1=tmp, op=AOP.add)
                    pend_late.append(assy)
                    # free AV psum via DMA, defer normalize into next pair
                    for h in range(2):
                        av = qs_av[h][1]
                        avs = sp.tile([65, 512], F32, tag=f"avs{h}", bufs=1)
                        nc.scalar.activation(out=avs, in_=av[0:65, :], func=AFT.Copy)
                        zrow = sp.tile([1, 512], F32, tag="zrow", bufs=1)
                        nc.vector.tensor_copy(zrow, avs[64:65, :])
                        zr32 = sp.tile([1, 512], F32, tag="zr32", bufs=1)
                        nc.vector.reciprocal_approx_fast(out=zr32, in_=zrow)
                        zrb = sp.tile([1, 512], BF16, tag=f"zrb{h}", bufs=1)
                        nc.vector.tensor_copy(zrb, zr32)

                        def norm(h=h, pr=pr, ic=ic, sl=sl, avs=avs, zrb=zrb):
                            zbc = pp.tile([128, 512], F32, tag=("qsA", "qsB")[h],
                                          bufs=1, name=f"zbc{h}")
                            nc.tensor.matmul(zbc[0:64, :], onesr1, zrb,
                                             start=True, stop=True,
                                             skip_group_check=True)
                            if h == 0:
                                nc.vector.tensor_tensor(out=onorm[0:64, pr, sl],
                                                        in0=avs[0:64, :],
                                                        in1=zbc[0:64, :], op=AOP.mult)
                            else:
                                otmp = sp.tile([64, 512], BF16, tag="zr32", bufs=1,
                                               name="otmp")
                                nc.vector.tensor_tensor(out=otmp, in0=avs[0:64, :],
                                                        in1=zbc[0:64, :], op=AOP.mult)
                                nc.sync.dma_start(out=onorm[64:128, pr, sl], in_=otmp)
                        pend_early.append(norm)
            emit(pend_early)
            emit(pend_late)

            # ---- phase 4: output projection ----
            for it in range(8):
                yp = pp.tile([128, 512], F32, tag=("dA", "dB")[it % 2], bufs=2)
                for prr in range(4):
                    nc.tensor.matmul(yp, onorm[:, prr, it * 128:(it + 1) * 128],
                                     wo[:, prr, :], start=(prr == 0), stop=(prr == 3))
                yt = sp.tile([128, DIM], F32, tag="qss", bufs=1)
                nc.vector.tensor_tensor(out=yt, in0=yp, in1=bb, op=AOP.add)
                nc.sync.dma_start(out=y_out[it * 128:(it + 1) * 128, :], in_=yt)

            # ---- score finalize ----
            for ic in range(2):
                sl = slice(ic * 512, (ic + 1) * 512)
                scp = pp.tile([1, 512], F32, tag=ttags[ic], bufs=1)
                nc.tensor.matmul(scp, ones_f32, sc_acc[:, sl], start=True, stop=True)
                nc.vector.scalar_tensor_tensor(out=sc_sb[:, sl], in0=scp, scalar=SCALE,
                                               in1=rnz[:, sl], op0=AOP.mult,
                                               op1=AOP.mult)
            nc.gpsimd.dma_start(out=sc_out[:, :], in_=sc_sb)
    nc.finalize()
    return nc


def _get_nc():
    if "nc" not in _cache:
        _cache["nc"] = _build()
    return _cache["nc"]


def _run_device(inputs, trace=False):
    x = np.asarray(inputs["x"], np.float32)
    cp_mask = np.asarray(inputs["cp_mask"])
    w_qkv = np.asarray(inputs["w_qkv"], np.float32)
    w_out = np.asarray(inputs["w_out"], np.float32)
    b_out = np.asarray(inputs["b_out"], np.float32)

    bf = mybir.dt.np(BF16)
    wqk = np.ascontiguousarray(w_qkv[:, :2 * INNER])
    whi = wqk.astype(bf)
    wlo = (wqk - whi.astype(np.float32)).astype(bf)
    wv = np.ascontiguousarray(w_qkv[:, 2 * INNER:]).astype(bf)
    wob = np.ascontiguousarray(w_out).astype(bf)
    maskT = np.ascontiguousarray(cp_mask.T).astype(bf)
    boutr = np.ascontiguousarray(b_out.reshape(1, DIM))
    idn = np.eye(128, dtype=bf)

    in_maps = []
    for b in range(B):
        xT = np.ascontiguousarray(x[b].T)
        xhi = xT.astype(bf)
        xlo = (xT - xhi.astype(np.float32)).astype(bf)
        in_maps.append({
            "xhi": xhi, "xlo": xlo,
            "whi": whi, "wlo": wlo,
            "wv": wv, "wo": wob,
            "maskT": maskT, "bout": boutr, "idn": idn,
        })

    nc = _get_nc()
    res = run_bass_kernel_spmd(nc, in_maps, core_ids=list(range(B)), trace=trace)
    y = np.stack([res.results[b]["y"] for b in range(B)])
    score = np.stack([res.results[b]["score"][0] for b in range(B)])
    return y, score, res


def _apply_swap(y, score, patches):
    idx = np.argsort(score, axis=-1, kind="stable")[::-1]
    out = y.copy()
    clone = y
    bi = np.arange(B)
    for i in range(1, patches + 1):
        ti = idx[:, i]
        out[bi, i] = clone[bi, ti]
        out[bi, ti] = clone[:, i]
    return out


def kernel(**inputs):
    patches = int(np.asarray(inputs["patches_in_core_nodes"]))
    y, score, _ = _run_device(inputs, trace=False)
    return _apply_swap(y, score, patches)


# revision 19
# speedup vs baseline: 1.7374x; 1.1481x over previous
"""CPAttention Trainium2 kernel: 8-way batch-data-parallel over 8 NeuronCores.

v2: head-pair processing with PE packing.
  - dots: fp32, two heads row-packed (K=64 at tile_position (0,0)/(64,0))
  - AV:   bf16, two heads col-packed into one [128,1024] PSUM (cols 0:64/64:128)
  - pack: 4-col-packed ones-matmuls -> score_A(row0, fp32), score_B(row32, fp32),
          Z_A(row64, bf16), Z_B(row96, bf16), accumulated over j-tiles
  - outproj: per-pair K=128 bf16
Score path (argsort-critical) stays fp32; softmax/output path is bf16.
Host applies the argsort + 16-step row swap (commutes with w_out).
"""
import numpy as np

import concourse.bacc as bacc
import concourse.tile as tile
from concourse import mybir
from concourse.bass_utils import run_bass_kernel_spmd

F32 = mybir.dt.float32
BF16 = mybir.dt.bfloat16
U32 = mybir.dt.uint32
AOP = mybir.AluOpType
AFT = mybir.ActivationFunctionType

B, N, DIM = 8, 1024, 512
HEADS, DH = 8, 64
INNER = 512
SCALE = DH ** -0.5

_cache = {}


def _emit_burst(nc, oTp, pack, vv, ones32, onesbf, pr, jt, es, abs_):
    first, last = (jt == 0), (jt == 7)
    for ic in range(2):
        sl = slice(ic * 512, (ic + 1) * 512)
        for hh in range(2):
            nc.tensor.matmul(
                oTp[hh * 64:(hh + 1) * 64, sl],
                vv[:, 2 * pr + hh, jt, :], es[hh][:, sl],
                start=first, stop=last,
                tile_position=(0, hh * 64),
                skip_group_check=True)
    for ic in range(2):
        sl = slice(ic * 512, (ic + 1) * 512)
        for hh in range(2):
            st = hh * 32 + ic * 64
            nc.tensor.matmul(
                pack[st:st + 1, sl],
                ones32, abs_[hh][:, sl],
                start=first, stop=last,
                tile_position=(0, st),
                skip_group_check=True)
    for ic in range(2):
        sl = slice(ic * 512, (ic + 1) * 512)
        for hh in range(2):
            st = hh * 32 + (1 - ic) * 64
            nc.tensor.matmul(
                pack[st:st + 1, sl],
                onesbf, es[hh][:, sl],
                start=first, stop=last,
                tile_position=(0, st),
                skip_group_check=True)


def _build():
    nc = bacc.Bacc()
    xhi_d = nc.declare_dram_parameter("xhi", [DIM, N], BF16, isOutput=False)
    xlo_d = nc.declare_dram_parameter("xlo", [DIM, N], BF16, isOutput=False)
    maskT = nc.declare_dram_parameter("maskT", [N, N], BF16, isOutput=False)
    whi_d = nc.declare_dram_parameter("whi", [DIM, 2 * INNER], BF16, isOutput=False)
    wlo_d = nc.declare_dram_parameter("wlo", [DIM, 2 * INNER], BF16, isOutput=False)
    wvbf = nc.declare_dram_parameter("wvbf", [DIM, INNER], BF16, isOutput=False)
    wobf = nc.declare_dram_parameter("wobf", [INNER, DIM], BF16, isOutput=False)
    bout = nc.declare_dram_parameter("bout", [1, DIM], F32, isOutput=False)
    y_out = nc.declare_dram_parameter("y", [N, DIM], F32, isOutput=True)
    sc_out = nc.declare_dram_parameter("score", [1, N], F32, isOutput=True)

    with tile.TileContext(nc) as tc:
        with tc.tile_pool(name="cst", bufs=1) as cst, \
             tc.tile_pool(name="wrk", bufs=3) as wrk, \
             tc.tile_pool(name="wrk4", bufs=4) as wrk4, \
             tc.tile_pool(name="eph", bufs=2) as eph, \
             tc.tile_pool(name="one", bufs=1) as one, \
             tc.tile_pool(name="ppA", bufs=1, space="PSUM") as ppA, \
             tc.tile_pool(name="ppB", bufs=1, space="PSUM") as ppB, \
             tc.tile_pool(name="poT", bufs=1, space="PSUM") as poT, \
             tc.tile_pool(name="ppk", bufs=1, space="PSUM") as ppk:

            # ---- loads ----
            xhi = cst.tile([128, 4, N], BF16)
            nc.sync.dma_start(out=xhi, in_=xhi_d[:, :].rearrange("(t p) i -> p t i", p=128))
            whi = cst.tile([128, 4, 2 * INNER], BF16)
            nc.sync.dma_start(out=whi, in_=whi_d[:, :].rearrange("(t p) c -> p t c", p=128))
            xlo = cst.tile([128, 4, N], BF16)
            nc.sync.dma_start(out=xlo, in_=xlo_d[:, :].rearrange("(t p) i -> p t i", p=128))
            wlo = cst.tile([128, 4, 2 * INNER], BF16)
            nc.sync.dma_start(out=wlo, in_=wlo_d[:, :].rearrange("(t p) c -> p t c", p=128))
            msk = cst.tile([128, 8, N], BF16)
            nc.sync.dma_start(out=msk, in_=maskT[:, :].rearrange("(t p) i -> p t i", p=128))
            wvb = cst.tile([128, 4, INNER], BF16)
            nc.sync.dma_start(out=wvb, in_=wvbf[:, :].rearrange("(t p) c -> p t c", p=128))
            wob = cst.tile([128, 4, DIM], BF16)
            nc.sync.dma_start(out=wob, in_=wobf[:, :].rearrange("(t p) e -> p t e", p=128))
            bb = cst.tile([128, DIM], F32)
            nc.sync.dma_start(out=bb, in_=bout[0:1, :].to_broadcast([128, DIM]))

            ones32 = cst.tile([128, 1], F32)
            nc.vector.memset(ones32, 1.0)
            onesbf = cst.tile([128, 1], BF16)
            nc.vector.memset(onesbf, 1.0)
            onesr1 = cst.tile([1, 128], BF16)
            nc.vector.memset(onesr1, 1.0)
            sel0 = cst.tile([128, 1], F32)
            nc.vector.memset(sel0, 0.0)
            nc.vector.memset(sel0[0:1, :], 1.0)
            nc.vector.memset(sel0[32:33, :], 1.0)
            sel1 = cst.tile([128, 1], F32)
            nc.vector.memset(sel1, 0.0)
            nc.vector.memset(sel1[64:65, :], 1.0)
            nc.vector.memset(sel1[96:97, :], 1.0)

            qkT = cst.tile([128, 8, N], F32)
            vv = cst.tile([128, HEADS, 8, DH], BF16)
            onorm = cst.tile([128, 4, N], BF16)
            sc_acc = cst.tile([128, N], F32)
            nc.vector.memset(sc_acc, 0.0)

            # ---- QKV q/k part (fp32) ----
            for ct in range(8):
                for ic in range(2):
                    qtag = "dA" if (ct * 2 + ic) % 2 == 0 else "dB"
                    qpool = ppA if qtag == "dA" else ppB
                    pq = qpool.tile([128, N], F32, tag=qtag)
                    cs = slice(ct * 128, (ct + 1) * 128)
                    isl = slice(ic * 512, (ic + 1) * 512)
                    for kt in range(4):
                        nc.tensor.matmul(pq[:, isl], whi[:, kt, cs],
                                         xhi[:, kt, isl], start=(kt == 0), stop=False)
                    for kt in range(4):
                        nc.tensor.matmul(pq[:, isl], whi[:, kt, cs],
                                         xlo[:, kt, isl], start=False, stop=False)
                    for kt in range(4):
                        nc.tensor.matmul(pq[:, isl], wlo[:, kt, cs],
                                         xhi[:, kt, isl], start=False, stop=(kt == 3))
                    nc.vector.tensor_copy(qkT[:, ct, ic * 512:(ic + 1) * 512],
                                          pq[:, ic * 512:(ic + 1) * 512])

            # ---- V part (bf16) ----
            for jt in range(8):
                vpool, vtag = (ppB, "dB") if jt % 2 == 0 else (ppA, "dA")
                pv = vpool.tile([128, N], F32, tag=vtag)
                for kt in range(4):
                    nc.tensor.matmul(
                        pv[:, 0:512],
                        xhi[:, kt, jt * 128:(jt + 1) * 128],
                        wvb[:, kt, :],
                        start=(kt == 0), stop=(kt == 3))
                nc.vector.tensor_copy(
                    vv[:, :, jt, :],
                    pv[:, 0:512].rearrange("p (h d) -> p h d", h=HEADS))

            # ---- nnz (needs only msk): compute early, off the tail ----
            nzp = ppB.tile([1, N], F32, tag="dB")
            for jt in range(8):
                for ic in range(2):
                    sl = slice(ic * 512, (ic + 1) * 512)
                    nc.tensor.matmul(nzp[0:1, sl], onesbf, msk[:, jt, sl],
                                     start=(jt == 0), stop=(jt == 7))
            scr = one.tile([1, N], F32, tag="scr")
            rnz = one.tile([1, N], F32, tag="rnz")
            nc.vector.reciprocal_approx_accurate(out=rnz, in_=nzp, scratch=scr)

            # ---- attention, head pairs ----
            for pr in range(4):
                hA, hB = 2 * pr, 2 * pr + 1
                oTp = poT.tile([128, N], F32, tag="oT")
                pack = ppk.tile([128, N], F32, tag="pk")
                carry = None
                for jt in range(8):
                    dA = ppA.tile([128, N], F32, tag="dA")
                    dB = ppB.tile([128, N], F32, tag="dB")
                    for ic in range(2):
                        nc.tensor.matmul(
                            dA[:, ic * 512:(ic + 1) * 512],
                            qkT[0:64, 4 + pr, jt * 128:(jt + 1) * 128],
                            qkT[0:64, pr, ic * 512:(ic + 1) * 512],
                            start=True, stop=True, tile_position=(0, 0))
                        nc.tensor.matmul(
                            dB[:, ic * 512:(ic + 1) * 512],
                            qkT[64:128, 4 + pr, jt * 128:(jt + 1) * 128],
                            qkT[64:128, pr, ic * 512:(ic + 1) * 512],
                            start=True, stop=True, tile_position=(64, 0))
                    if carry is not None:
                        _emit_burst(nc, oTp, pack, vv, ones32, onesbf, pr, *carry)
                    es, abs_ = [], []
                    for hh, dots in ((0, dA), (1, dB)):
                        t = wrk.tile([128, N], F32, tag="t")
                        nc.vector.tensor_tensor(out=t, in0=dots, in1=msk[:, jt, :],
                                                op=AOP.mult)
                        e = wrk4.tile([128, N], BF16, tag="e")
                        nc.scalar.activation(out=e, in_=t, func=AFT.Exp, scale=SCALE)
                        ab = wrk4.tile([128, N], F32, tag="ab")
                        nc.vector.tensor_scalar(
                            out=ab.bitcast(U32), in0=t.bitcast(U32),
                            scalar1=0x7FFFFFFF, scalar2=None, op0=AOP.bitwise_and)
                        es.append(e)
                        abs_.append(ab)
                    carry = (jt, es, abs_)
                _emit_burst(nc, oTp, pack, vv, ones32, onesbf, pr, *carry)
                # harvest: score cells r0/r32 (ic0) + r64/r96 (ic1) -> sc_acc
                nc.vector.tensor_tensor(out=sc_acc[0:97, :], in0=sc_acc[0:97, :],
                                        in1=pack[0:97, :], op=AOP.add)
                # Z_A = {row64 ic0, row0 ic1}; Z_B = {row96 ic0, row32 ic1}
                zshift = eph.tile([128, 2, N], BF16, tag="zsh")
                zrow = eph.tile([1, 2, N], BF16, tag="zrow")
                nc.scalar.activation(out=zshift[64:65, 0, 0:512],
                                     in_=pack[64:65, 0:512], func=AFT.Copy)
                nc.scalar.activation(out=zrow[0:1, 0, 512:1024],
                                     in_=pack[0:1, 512:1024], func=AFT.Copy)
                nc.scalar.activation(out=zshift[96:97, 1, 0:512],
                                     in_=pack[96:97, 0:512], func=AFT.Copy)
                nc.scalar.activation(out=zshift[32:33, 1, 512:1024],
                                     in_=pack[32:33, 512:1024], func=AFT.Copy)
                # partition shifts to row 0 via SBUF->SBUF DMA
                nc.sync.dma_start(out=zrow[0:1, 0, 0:512], in_=zshift[64:65, 0, 0:512])
                nc.sync.dma_start(out=zrow[0:1, 1, 0:512], in_=zshift[96:97, 1, 0:512])
                nc.sync.dma_start(out=zrow[0:1, 1, 512:1024],
                                  in_=zshift[32:33, 1, 512:1024])
                # broadcast Z over partitions: rows 0:64 = Z_A, 64:128 = Z_B
                zbc = ppk.tile([128, N], F32, tag="pk")
                for ic in range(2):
                    sl = slice(ic * 512, (ic + 1) * 512)
                    nc.tensor.matmul(zbc[0:64, sl], onesr1[:, 0:64],
                                     zrow[0:1, 0, sl],
                                     start=True, stop=True, tile_position=(0, 0))
                    nc.tensor.matmul(zbc[64:128, sl], onesr1[:, 0:64],
                                     zrow[0:1, 1, sl],
                                     start=True, stop=True, tile_position=(0, 64))
                zr = eph.tile([128, N], F32, tag="zr")
                nc.vector.reciprocal_approx_fast(out=zr, in_=zbc)
                nc.vector.tensor_tensor(out=onorm[:, pr, :], in0=oTp, in1=zr,
                                        op=AOP.mult)
                if pr == 3:
                    scp = ppB.tile([1, N], F32, tag="dB")
                    nc.tensor.matmul(scp[0:1, 0:512], sel0, sc_acc[:, 0:512],
                                     start=True, stop=True)
                    nc.tensor.matmul(scp[0:1, 512:1024], sel1,
                                     sc_acc[:, 512:1024], start=True, stop=True)

            # ---- output projection (per pair, K=128) ----
            for it in range(8):
                ypool, ytag = (ppA, "dA") if it % 2 == 0 else (ppB, "dB")
                yp = ypool.tile([128, N], F32, tag=ytag)
                for pr in range(4):
                    nc.tensor.matmul(
                        yp[:, 0:512],
                        onorm[:, pr, it * 128:(it + 1) * 128],
                        wob[:, pr, :],
                        start=(pr == 0), stop=(pr == 3))
                yt = eph.tile([128, DIM], F32, tag="yt")
                nc.vector.tensor_tensor(out=yt, in0=yp[:, 0:512], in1=bb, op=AOP.add)
                nc.sync.dma_start(out=y_out[it * 128:(it + 1) * 128, :], in_=yt)

            # ---- score: sum the 8 per-head rows, / nnz, * scale ----
            sc_sb = one.tile([1, N], F32, tag="scs")
            nc.vector.scalar_tensor_tensor(
                out=sc_sb, in0=scp, scalar=SCALE, in1=rnz,
                op0=AOP.mult, op1=AOP.mult)

            # ---- outputs ----
            nc.gpsimd.dma_start(out=sc_out[:, :], in_=sc_sb)
    nc.finalize()
    return nc


def _get_nc():
    if "nc" not in _cache:
        _cache["nc"] = _build()
    return _cache["nc"]


def _run_device(inputs, trace=False):
    x = np.asarray(inputs["x"], np.float32)
    cp_mask = np.asarray(inputs["cp_mask"])
    w_qkv = np.asarray(inputs["w_qkv"], np.float32)
    w_out = np.asarray(inputs["w_out"], np.float32)
    b_out = np.asarray(inputs["b_out"], np.float32)

    bf = mybir.dt.np(BF16)
    maskT = np.ascontiguousarray(cp_mask.T).astype(bf)
    wqk = np.ascontiguousarray(w_qkv[:, :2 * INNER])
    whi = wqk.astype(bf)
    wlo = (wqk - whi.astype(np.float32)).astype(bf)
    wvbf = np.ascontiguousarray(w_qkv[:, 2 * INNER:]).astype(bf)
    wobf = np.ascontiguousarray(w_out).astype(bf)
    boutr = np.ascontiguousarray(b_out.reshape(1, DIM))

    in_maps = []
    for b in range(B):
        xTb = np.ascontiguousarray(x[b].T)
        xhi = xTb.astype(bf)
        xlo = (xTb - xhi.astype(np.float32)).astype(bf)
        in_maps.append({
            "xhi": xhi,
            "xlo": xlo,
            "maskT": maskT,
            "whi": whi,
            "wlo": wlo,
            "wvbf": wvbf,
            "wobf": wobf,
            "bout": boutr,
        })

    nc = _get_nc()
    res = run_bass_kernel_spmd(nc, in_maps, core_ids=list(range(B)), trace=trace)
    y = np.stack([res.results[b]["y"] for b in range(B)])
    score = np.stack([res.results[b]["score"][0] for b in range(B)])
    return y, score, res


def _apply_swap(y, score, patches):
    idx = np.argsort(score, axis=-1, kind="stable")[::-1]
    out = y.copy()
    clone = y
    bi = np.arange(B)
    for i in range(1, patches + 1):
        ti = idx[:, i]
        out[bi, i] = clone[bi, ti]
        out[bi, ti] = clone[:, i]
    return out


def kernel(**inputs):
    patches = int(np.asarray(inputs["patches_in_core_nodes"]))
    y, score, _ = _run_device(inputs, trace=False)
    return _apply_swap(y, score, patches)



# revision 20
# speedup vs baseline: 1.7598x; 1.0129x over previous
"""CPAttention Trainium2 kernel: 8-way batch-data-parallel over 8 NeuronCores.

v2: head-pair processing with PE packing.
  - dots: fp32, two heads row-packed (K=64 at tile_position (0,0)/(64,0))
  - AV:   bf16, two heads col-packed into one [128,1024] PSUM (cols 0:64/64:128)
  - pack: 4-col-packed ones-matmuls -> score_A(row0, fp32), score_B(row32, fp32),
          Z_A(row64, bf16), Z_B(row96, bf16), accumulated over j-tiles
  - outproj: per-pair K=128 bf16
Score path (argsort-critical) stays fp32; softmax/output path is bf16.
Host applies the argsort + 16-step row swap (commutes with w_out).
"""
import numpy as np

import concourse.bacc as bacc
import concourse.tile as tile
from concourse import mybir
from concourse.bass_utils import run_bass_kernel_spmd

F32 = mybir.dt.float32
BF16 = mybir.dt.bfloat16
U32 = mybir.dt.uint32
AOP = mybir.AluOpType
AFT = mybir.ActivationFunctionType

B, N, DIM = 8, 1024, 512
HEADS, DH = 8, 64
INNER = 512
SCALE = DH ** -0.5

_cache = {}


def _emit_burst(nc, oTp, pack, vv, ones32, onesbf, pr, jt, es, abs_):
    first, last = (jt == 0), (jt == 7)
    for ic in range(2):
        sl = slice(ic * 512, (ic + 1) * 512)
        for hh in range(2):
            nc.tensor.matmul(
                oTp[hh * 64:(hh + 1) * 64, sl],
                vv[:, 2 * pr + hh, jt, :], es[hh][:, sl],
                start=first, stop=last,
                tile_position=(0, hh * 64),
                skip_group_check=True)
    for ic in range(2):
        sl = slice(ic * 512, (ic + 1) * 512)
        for hh in range(2):
            st = hh * 32 + ic * 64
            nc.tensor.matmul(
                pack[st:st + 1, sl],
                ones32, abs_[hh][:, sl],
                start=first, stop=last,
                tile_position=(0, st),
                skip_group_check=True)
    for ic in range(2):
        sl = slice(ic * 512, (ic + 1) * 512)
        for hh in range(2):
            st = hh * 32 + (1 - ic) * 64
            nc.tensor.matmul(
                pack[st:st + 1, sl],
                onesbf, es[hh][:, sl],
                start=first, stop=last,
                tile_position=(0, st),
                skip_group_check=True)


def _build():
    nc = bacc.Bacc()
    xhi_d = nc.declare_dram_parameter("xhi", [DIM, N], BF16, isOutput=False)
    xlo_d = nc.declare_dram_parameter("xlo", [DIM, N], BF16, isOutput=False)
    maskT = nc.declare_dram_parameter("maskT", [N, N], BF16, isOutput=False)
    whi_d = nc.declare_dram_parameter("whi", [DIM, 2 * INNER], BF16, isOutput=False)
    wlo_d = nc.declare_dram_parameter("wlo", [DIM, 2 * INNER], BF16, isOutput=False)
    wvbf = nc.declare_dram_parameter("wvbf", [DIM, INNER], BF16, isOutput=False)
    wobf = nc.declare_dram_parameter("wobf", [INNER, DIM], BF16, isOutput=False)
    bout = nc.declare_dram_parameter("bout", [1, DIM], F32, isOutput=False)
    y_out = nc.declare_dram_parameter("y", [N, DIM], F32, isOutput=True)
    sc_out = nc.declare_dram_parameter("score", [1, N], F32, isOutput=True)

    with tile.TileContext(nc) as tc:
        with tc.tile_pool(name="cst", bufs=1) as cst, \
             tc.tile_pool(name="wrk", bufs=3) as wrk, \
             tc.tile_pool(name="wrk4", bufs=4) as wrk4, \
             tc.tile_pool(name="eph", bufs=2) as eph, \
             tc.tile_pool(name="one", bufs=1) as one, \
             tc.tile_pool(name="ppA", bufs=1, space="PSUM") as ppA, \
             tc.tile_pool(name="ppB", bufs=1, space="PSUM") as ppB, \
             tc.tile_pool(name="poT", bufs=1, space="PSUM") as poT, \
             tc.tile_pool(name="ppk", bufs=1, space="PSUM") as ppk:

            # ---- loads ----
            xhi = cst.tile([128, 4, N], BF16)
            nc.sync.dma_start(out=xhi, in_=xhi_d[:, :].rearrange("(t p) i -> p t i", p=128))
            whi = cst.tile([128, 4, 2 * INNER], BF16)
            nc.sync.dma_start(out=whi, in_=whi_d[:, :].rearrange("(t p) c -> p t c", p=128))
            xlo = cst.tile([128, 4, N], BF16)
            nc.sync.dma_start(out=xlo, in_=xlo_d[:, :].rearrange("(t p) i -> p t i", p=128))
            wlo = cst.tile([128, 4, 2 * INNER], BF16)
            nc.sync.dma_start(out=wlo, in_=wlo_d[:, :].rearrange("(t p) c -> p t c", p=128))
            msk = cst.tile([128, 8, N], BF16)
            nc.sync.dma_start(out=msk, in_=maskT[:, :].rearrange("(t p) i -> p t i", p=128))
            wvb = cst.tile([128, 4, INNER], BF16)
            nc.sync.dma_start(out=wvb, in_=wvbf[:, :].rearrange("(t p) c -> p t c", p=128))
            wob = cst.tile([128, 4, DIM], BF16)
            nc.sync.dma_start(out=wob, in_=wobf[:, :].rearrange("(t p) e -> p t e", p=128))
            bb = cst.tile([128, DIM], F32)
            nc.sync.dma_start(out=bb, in_=bout[0:1, :].to_broadcast([128, DIM]))

            ones32 = cst.tile([128, 1], F32)
            nc.vector.memset(ones32, 1.0)
            onesbf = cst.tile([128, 1], BF16)
            nc.vector.memset(onesbf, 1.0)
            onesr1 = cst.tile([1, 128], BF16)
            nc.vector.memset(onesr1, 1.0)
            sel0 = cst.tile([128, 1], F32)
            nc.vector.memset(sel0, 0.0)
            nc.vector.memset(sel0[0:1, :], 1.0)
            nc.vector.memset(sel0[32:33, :], 1.0)
            sel1 = cst.tile([128, 1], F32)
            nc.vector.memset(sel1, 0.0)
            nc.vector.memset(sel1[64:65, :], 1.0)
            nc.vector.memset(sel1[96:97, :], 1.0)

            qhi = cst.tile([128, 4, N], BF16)
            qlo = cst.tile([128, 4, N], BF16)
            khi = cst.tile([128, 4, N], BF16)
            klo = cst.tile([128, 4, N], BF16)
            vv = cst.tile([128, HEADS, 8, DH], BF16)
            onorm = cst.tile([128, 4, N], BF16)
            sc_acc = cst.tile([128, N], F32)
            nc.vector.memset(sc_acc, 0.0)

            # ---- QKV q/k part (fp32) ----
            for ct in range(8):
                for ic in range(2):
                    qtag = "dA" if (ct * 2 + ic) % 2 == 0 else "dB"
                    qpool = ppA if qtag == "dA" else ppB
                    pq = qpool.tile([128, N], F32, tag=qtag)
                    cs = slice(ct * 128, (ct + 1) * 128)
                    isl = slice(ic * 512, (ic + 1) * 512)
                    for kt in range(4):
                        nc.tensor.matmul(pq[:, isl], whi[:, kt, cs],
                                         xhi[:, kt, isl], start=(kt == 0), stop=False)
                    for kt in range(4):
                        nc.tensor.matmul(pq[:, isl], whi[:, kt, cs],
                                         xlo[:, kt, isl], start=False, stop=False)
                    for kt in range(4):
                        nc.tensor.matmul(pq[:, isl], wlo[:, kt, cs],
                                         xhi[:, kt, isl], start=False, stop=(kt == 3))
                    hi, lo = (qhi, qlo) if ct < 4 else (khi, klo)
                    c4 = ct % 4
                    nc.scalar.activation(out=hi[:, c4, isl], in_=pq[:, isl],
                                         func=AFT.Copy)
                    nc.vector.tensor_tensor(out=lo[:, c4, isl], in0=pq[:, isl],
                                            in1=hi[:, c4, isl], op=AOP.subtract)

            # ---- V part (bf16) ----
            for jt in range(8):
                vpool, vtag = (ppB, "dB") if jt % 2 == 0 else (ppA, "dA")
                pv = vpool.tile([128, N], F32, tag=vtag)
                for kt in range(4):
                    nc.tensor.matmul(
                        pv[:, 0:512],
                        xhi[:, kt, jt * 128:(jt + 1) * 128],
                        wvb[:, kt, :],
                        start=(kt == 0), stop=(kt == 3))
                nc.vector.tensor_copy(
                    vv[:, :, jt, :],
                    pv[:, 0:512].rearrange("p (h d) -> p h d", h=HEADS))

            # ---- nnz (needs only msk): compute early, off the tail ----
            nzp = ppB.tile([1, N], F32, tag="dB")
            for jt in range(8):
                for ic in range(2):
                    sl = slice(ic * 512, (ic + 1) * 512)
                    nc.tensor.matmul(nzp[0:1, sl], onesbf, msk[:, jt, sl],
                                     start=(jt == 0), stop=(jt == 7))
            scr = one.tile([1, N], F32, tag="scr")
            rnz = one.tile([1, N], F32, tag="rnz")
            nc.vector.reciprocal_approx_accurate(out=rnz, in_=nzp, scratch=scr)

            # ---- attention, head pairs ----
            for pr in range(4):
                hA, hB = 2 * pr, 2 * pr + 1
                oTp = poT.tile([128, N], F32, tag="oT")
                pack = ppk.tile([128, N], F32, tag="pk")
                carry = None
                for jt in range(8):
                    dA = ppA.tile([128, N], F32, tag="dA")
                    dB = ppB.tile([128, N], F32, tag="dB")
                    js = slice(jt * 128, (jt + 1) * 128)
                    for ic in range(2):
                        isl = slice(ic * 512, (ic + 1) * 512)
                        terms = ((khi, qhi), (klo, qhi), (khi, qlo))
                        for tix, (st, mv) in enumerate(terms):
                            nc.tensor.matmul(
                                dA[:, isl], st[0:64, pr, js], mv[0:64, pr, isl],
                                start=(tix == 0), stop=(tix == 2),
                                tile_position=(0, 0), skip_group_check=True)
                            nc.tensor.matmul(
                                dB[:, isl], st[64:128, pr, js], mv[64:128, pr, isl],
                                start=(tix == 0), stop=(tix == 2),
                                tile_position=(64, 0), skip_group_check=True)
                    if carry is not None:
                        _emit_burst(nc, oTp, pack, vv, ones32, onesbf, pr, *carry)
                    es, abs_ = [], []
                    for hh, dots in ((0, dA), (1, dB)):
                        t = wrk.tile([128, N], F32, tag="t")
                        nc.vector.tensor_tensor(out=t, in0=dots, in1=msk[:, jt, :],
                                                op=AOP.mult)
                        e = wrk4.tile([128, N], BF16, tag="e")
                        nc.scalar.activation(out=e, in_=t, func=AFT.Exp, scale=SCALE)
                        ab = wrk4.tile([128, N], F32, tag="ab")
                        nc.vector.tensor_scalar(
                            out=ab.bitcast(U32), in0=t.bitcast(U32),
                            scalar1=0x7FFFFFFF, scalar2=None, op0=AOP.bitwise_and)
                        es.append(e)
                        abs_.append(ab)
                    carry = (jt, es, abs_)
                _emit_burst(nc, oTp, pack, vv, ones32, onesbf, pr, *carry)
                # harvest: score cells r0/r32 (ic0) + r64/r96 (ic1) -> sc_acc
                nc.vector.tensor_tensor(out=sc_acc[0:97, :], in0=sc_acc[0:97, :],
                                        in1=pack[0:97, :], op=AOP.add)
                # Z_A = {row64 ic0, row0 ic1}; Z_B = {row96 ic0, row32 ic1}
                zshift = eph.tile([128, 2, N], BF16, tag="zsh")
                zrow = eph.tile([1, 2, N], BF16, tag="zrow")
                nc.scalar.activation(out=zshift[64:65, 0, 0:512],
                                     in_=pack[64:65, 0:512], func=AFT.Copy)
                nc.scalar.activation(out=zrow[0:1, 0, 512:1024],
                                     in_=pack[0:1, 512:1024], func=AFT.Copy)
                nc.scalar.activation(out=zshift[96:97, 1, 0:512],
                                     in_=pack[96:97, 0:512], func=AFT.Copy)
                nc.scalar.activation(out=zshift[32:33, 1, 512:1024],
                                     in_=pack[32:33, 512:1024], func=AFT.Copy)
                # partition shifts to row 0 via SBUF->SBUF DMA
                nc.sync.dma_start(out=zrow[0:1, 0, 0:512], in_=zshift[64:65, 0, 0:512])
                nc.sync.dma_start(out=zrow[0:1, 1, 0:512], in_=zshift[96:97, 1, 0:512])
                nc.sync.dma_start(out=zrow[0:1, 1, 512:1024],
                                  in_=zshift[32:33, 1, 512:1024])
                # broadcast Z over partitions: rows 0:64 = Z_A, 64:128 = Z_B
                zbc = ppk.tile([128, N], F32, tag="pk")
                for ic in range(2):
                    sl = slice(ic * 512, (ic + 1) * 512)
                    nc.tensor.matmul(zbc[0:64, sl], onesr1[:, 0:64],
                                     zrow[0:1, 0, sl],
                                     start=True, stop=True, tile_position=(0, 0))
                    nc.tensor.matmul(zbc[64:128, sl], onesr1[:, 0:64],
                                     zrow[0:1, 1, sl],
                                     start=True, stop=True, tile_position=(0, 64))
                zr = eph.tile([128, N], F32, tag="zr")
                nc.vector.reciprocal_approx_fast(out=zr, in_=zbc)
                nc.vector.tensor_tensor(out=onorm[:, pr, :], in0=oTp, in1=zr,
                                        op=AOP.mult)
                if pr == 3:
                    scp = ppB.tile([1, N], F32, tag="dB")
                    nc.tensor.matmul(scp[0:1, 0:512], sel0, sc_acc[:, 0:512],
                                     start=True, stop=True)
                    nc.tensor.matmul(scp[0:1, 512:1024], sel1,
                                     sc_acc[:, 512:1024], start=True, stop=True)

            # ---- output projection (per pair, K=128) ----
            for it in range(8):
                ypool, ytag = (ppA, "dA") if it % 2 == 0 else (ppB, "dB")
                yp = ypool.tile([128, N], F32, tag=ytag)
                for pr in range(4):
                    nc.tensor.matmul(
                        yp[:, 0:512],
                        onorm[:, pr, it * 128:(it + 1) * 128],
                        wob[:, pr, :],
                        start=(pr == 0), stop=(pr == 3))
                yt = eph.tile([128, DIM], F32, tag="yt")
                nc.vector.tensor_tensor(out=yt, in0=yp[:, 0:512], in1=bb, op=AOP.add)
                nc.sync.dma_start(out=y_out[it * 128:(it + 1) * 128, :], in_=yt)

            # ---- score: sum the 8 per-head rows, / nnz, * scale ----
            sc_sb = one.tile([1, N], F32, tag="scs")
            nc.vector.scalar_tensor_tensor(
                out=sc_sb, in0=scp, scalar=SCALE, in1=rnz,
                op0=AOP.mult, op1=AOP.mult)

            # ---- outputs ----
            nc.gpsimd.dma_start(out=sc_out[:, :], in_=sc_sb)
    nc.finalize()
    return nc


def _get_nc():
    if "nc" not in _cache:
        _cache["nc"] = _build()
    return _cache["nc"]


def _run_device(inputs, trace=False):
    x = np.asarray(inputs["x"], np.float32)
    cp_mask = np.asarray(inputs["cp_mask"])
    w_qkv = np.asarray(inputs["w_qkv"], np.float32)
    w_out = np.asarray(inputs["w_out"], np.float32)
    b_out = np.asarray(inputs["b_out"], np.float32)

    bf = mybir.dt.np(BF16)
    maskT = np.ascontiguousarray(cp_mask.T).astype(bf)
    wqk = np.ascontiguousarray(w_qkv[:, :2 * INNER])
    whi = wqk.astype(bf)
    wlo = (wqk - whi.astype(np.float32)).astype(bf)
    wvbf = np.ascontiguousarray(w_qkv[:, 2 * INNER:]).astype(bf)
    wobf = np.ascontiguousarray(w_out).astype(bf)
    boutr = np.ascontiguousarray(b_out.reshape(1, DIM))

    in_maps = []
    for b in range(B):
        xTb = np.ascontiguousarray(x[b].T)
        xhi = xTb.astype(bf)
        xlo = (xTb - xhi.astype(np.float32)).astype(bf)
        in_maps.append({
            "xhi": xhi,
            "xlo": xlo,
            "maskT": maskT,
            "whi": whi,
            "wlo": wlo,
            "wvbf": wvbf,
            "wobf": wobf,
            "bout": boutr,
        })

    nc = _get_nc()
    res = run_bass_kernel_spmd(nc, in_maps, core_ids=list(range(B)), trace=trace)
    y = np.stack([res.results[b]["y"] for b in range(B)])
    score = np.stack([res.results[b]["score"][0] for b in range(B)])
    return y, score, res


def _apply_swap(y, score, patches):
    idx = np.argsort(score, axis=-1, kind="stable")[::-1]
    out = y.copy()
    clone = y
    bi = np.arange(B)
    for i in range(1, patches + 1):
        ti = idx[:, i]
        out[bi, i] = clone[bi, ti]
        out[bi, ti] = clone[:, i]
    return out


def kernel(**inputs):
    patches = int(np.asarray(inputs["patches_in_core_nodes"]))
    y, score, _ = _run_device(inputs, trace=False)
    return _apply_swap(y, score, patches)

